# revision 1
# baseline (speedup 1.0000x reference)
"""Trainium2 Bass kernel for nn_FCGFAutoencoder (segment_max -> 3-layer MLP decoder).

Strategy (data-parallel over segments, per sharding hint):
  - batch_ids are sorted, so the host finds the 65 segment boundaries with
    searchsorted and repacks features into a [B, cap, C] array padded with
    -3e38 (max-identity).  Each of the 8 cores gets 8 whole segments.
  - Layout per (segment, core): partition p holds rows [p*L, (p+1)*L) of the
    segment -> each DMA is a single huge contiguous read (~2MB per quarter
    segment), which is required to reach HBM roofline (~358 GB/s/core).
  - On-device: per quarter-segment tile [128, (L/4)*32], a single strided
    reduce_max over the row axis gives [128, 32]; 3 tensor_max combines,
    a PE transpose to [32, 128], and a final reduce_max yield gT[:, s].
  - Decoder (replicated tiny MLP): h1T = relu(W1^T gT + b1), h2T = relu(W2^T
    h1T + b2), out = h2^T W3 + b3, all on PE/ACT/DVE, single [8, 3072] store.
  - Host concatenates the 8 per-core [8, 3072] outputs -> [64, 3, 1024].
"""

import os
import sys
import types

sys.path.insert(0, "/opt/trn_rl_repo")

import numpy as np


def _ensure_axon_hooks():
    """Some images lack antenv.axon_hooks; bass_utils imports it when
    trace=True under axon. Install a shim that lazily wires the real
    ctypes-based NTFF hook from trn_agent_boot if present, else degrades
    to no-trace instead of crashing."""
    try:
        import antenv.axon_hooks  # noqa: F401

        return
    except ImportError:
        pass
    try:
        import antenv
    except ImportError:
        return
    mod = types.ModuleType("antenv.axon_hooks")
    _hook = [None]

    def set_axon_ntff_profile_hook(h):
        _hook[0] = h

    def get_axon_ntff_profile_hook():
        if _hook[0] is None:
            try:
                from trn_agent_boot.trn_boot import _ntff_profile_via_ctypes

                _hook[0] = _ntff_profile_via_ctypes("/opt/axon/libaxon_pjrt.so")
            except Exception:
                return None
        return _hook[0]

    mod.set_axon_ntff_profile_hook = set_axon_ntff_profile_hook
    mod.get_axon_ntff_profile_hook = get_axon_ntff_profile_hook
    sys.modules["antenv.axon_hooks"] = mod
    antenv.axon_hooks = mod

N = 4_194_304
C = 32
B = 64
NUM_POINTS = 1024
NCORES = 8
SPC = B // NCORES  # segments per core
P = 128
J = 4  # DMA chunks per segment
NEG = -3.0e38
H1, H2, OUT_D = 256, 512, 3 * NUM_POINTS
K1, K2, NT = H1 // P, H2 // P, OUT_D // 512

LAST_RESULTS = None

_build_cache = {}


def _build(cap):
    if cap in _build_cache:
        return _build_cache[cap]

    import concourse.bacc as bacc
    import concourse.tile as tile
    from concourse import mybir
    from concourse.masks import make_identity
    from contextlib import ExitStack

    L = cap // P  # rows per partition per segment
    LQ = L // J  # rows per partition per DMA chunk
    F = LQ * C  # free elems per chunk tile

    f32 = mybir.dt.float32
    AX = mybir.AxisListType.X
    nc = bacc.Bacc("TRN2", target_bir_lowering=False)

    feats = nc.dram_tensor("feats", [SPC * cap, C], f32, kind="ExternalInput")
    w1 = nc.dram_tensor("w1", [C, H1], f32, kind="ExternalInput")
    b1t = nc.dram_tensor("b1t", [P, K1], f32, kind="ExternalInput")
    w2 = nc.dram_tensor("w2", [H1, H2], f32, kind="ExternalInput")
    b2t = nc.dram_tensor("b2t", [P, K2], f32, kind="ExternalInput")
    w3 = nc.dram_tensor("w3", [H2, OUT_D], f32, kind="ExternalInput")
    b3r = nc.dram_tensor("b3r", [SPC, OUT_D], f32, kind="ExternalInput")
    out = nc.dram_tensor("out", [SPC, OUT_D], f32, kind="ExternalOutput")

    # rows: s*cap + p*L + j*LQ + i  ->  [s, j, p, (i c)]
    fview = feats[:].rearrange("(s p j i) c -> s j p (i c)", s=SPC, p=P, j=J)

    with ExitStack() as ctx:
        tc = ctx.enter_context(tile.TileContext(nc))
        consts = ctx.enter_context(tc.tile_pool(name="consts", bufs=1))
        fpool = ctx.enter_context(tc.tile_pool(name="feat", bufs=6))
        outp = ctx.enter_context(tc.tile_pool(name="outp", bufs=2))
        redp = ctx.enter_context(tc.tile_pool(name="red", bufs=2 * J))
        ptr = ctx.enter_context(tc.tile_pool(name="ptr", bufs=2, space="PSUM"))
        pmm = ctx.enter_context(tc.tile_pool(name="pmm", bufs=2, space="PSUM"))
        pout = ctx.enter_context(tc.tile_pool(name="pout", bufs=2, space="PSUM"))

        ident = consts.tile([P, P], f32)
        make_identity(nc, ident)

        # weight/bias loads on the SP HWDGE ring; feature streaming runs on
        # the Act ring (so the ACT observer copies below share its engine).
        # biases first: the Act-ring observer copies below wait on these
        # lanes, and SP-ring DMAs are FIFO -- queueing them behind 6.5MB of
        # weights would stall the feature stream start by ~20us.
        b1_sb = consts.tile([P, K1], f32)
        nc.sync.dma_start(out=b1_sb, in_=b1t[:])
        b2_sb = consts.tile([P, K2], f32)
        nc.sync.dma_start(out=b2_sb, in_=b2t[:])
        HS = SPC // 2  # segments per decoder half
        b3_sb = []
        for h in range(2):
            bh = consts.tile([HS, OUT_D], f32, tag=f"b3h{h}")
            nc.sync.dma_start(out=bh, in_=b3r[h * HS : (h + 1) * HS])
            b3_sb.append(bh)
        w1_sb = consts.tile([C, H1], f32)
        nc.sync.dma_start(out=w1_sb, in_=w1[:])
        w2_sb = consts.tile([P, K1, H2], f32)
        nc.sync.dma_start(out=w2_sb, in_=w2[:].rearrange("(k p) n -> p k n", p=P))
        w3_sb = consts.tile([P, K2, OUT_D], f32)
        nc.sync.dma_start(out=w3_sb, in_=w3[:].rearrange("(k p) n -> p k n", p=P))

        # Observer copies: each engine may carry only ONE semaphore wait per
        # instruction, so advance ACT's and DVE's vector clocks over the
        # bias-load DMA lanes early; the decoder relu/add ops then need only
        # their PE wait.
        obs = consts.tile([1, 16], f32)
        nc.vector.tensor_copy(out=obs[0:1, 3:4], in_=b3_sb[0][0:1, 0:1])
        nc.vector.tensor_copy(out=obs[0:1, 4:5], in_=b3_sb[1][0:1, 0:1])

        # PE (Matmult/LDW) supports only ONE sync wait per instruction, so a
        # matmul whose inputs come from two unobserved semaphores fails to
        # compile. Prime PE with throwaway single-wait transposes so it has
        # observed the identity (Pool) and each weight-DMA lane before the
        # real matmuls. Each gets its own PSUM slot (slot reuse would add a
        # second, PE-release wait); the pool closes before the others open.
        with tc.tile_pool(name="prime", bufs=1, space="PSUM") as primep:
            prime_srcs = (
                ident[:, 0:C],
                w1_sb[:, 0:C],
                w2_sb[:, 0, 0:C],
                w3_sb[:, 0, 0:C],
            )
            pp = primep.tile([C, P], f32, tag="prime")
            for src in prime_srcs:
                kk = src.shape[0]
                nc.tensor.transpose(
                    out=pp[0:C, 0:kk], in_=src, identity=ident[0:kk, 0:kk]
                )

        gT = consts.tile([C, SPC], f32)
        segobs = consts.tile([1, SPC], f32)

        RB = 8  # row-blocks kept per chunk; small levels are overhead-bound

        def chunk_tree(eng, ft, scr, rj):
            # contiguous tree max over the row axis: pairs (i, c) with
            # (i + n/2, c); ping-pong between ft and scratch. Stops at RB
            # blocks (tail levels are fixed-overhead-dominated); rj is
            # [P, RB*C] and the cross-chunk combine finishes the job.
            cur, nxt = ft, ft
            n = LQ
            while n > 2 * RB:
                if n % 2 == 1:
                    eng.tensor_max(
                        cur[:, 0:C], cur[:, 0:C], cur[:, (n - 1) * C : n * C]
                    )
                    n -= 1
                half = n // 2
                eng.tensor_max(
                    nxt[:, 0 : half * C],
                    cur[:, 0 : half * C],
                    cur[:, half * C : n * C],
                )
                cur, nxt = nxt, cur
                n = half
            while n % RB:
                eng.tensor_max(cur[:, 0:C], cur[:, 0:C], cur[:, (n - 1) * C : n * C])
                n -= 1
            eng.tensor_max(
                rj[:, :], cur[:, 0 : (n // 2) * C], cur[:, (n // 2) * C : n * C]
            )

        def decode_half(h):
            # decoder for segments [h*HS, (h+1)*HS): runs while the other
            # half is still streaming, so only the last half is tail time.
            cols = slice(h * HS, (h + 1) * HS)
            # empty segments: reference maps -inf -> 0; padding is -3e38, so
            # mask = (g > -1e37) in {0,1}; g * mask zeroes empties exactly.
            mask = consts.tile([C, HS], f32, tag=f"mask{h}")
            gfix = consts.tile([C, HS], f32, tag=f"gfix{h}")
            nc.vector.tensor_scalar(
                out=mask[:, :],
                in0=gT[:, cols],
                scalar1=-1.0e37,
                scalar2=None,
                op0=mybir.AluOpType.is_gt,
            )
            nc.vector.tensor_mul(gfix[:, :], gT[:, cols], mask[:, :])

            # h1T[m] = relu(W1[:, m]^T @ g + b1[m])   [128, HS] per chunk m
            h1_sb = consts.tile([P, K1, HS], f32, tag=f"h1{h}")
            for m in range(K1):
                pm = pmm.tile([P, HS], f32, tag="pm")
                nc.tensor.matmul(
                    pm[:, :],
                    w1_sb[:, m * P : (m + 1) * P],
                    gfix[:, :],
                    start=True,
                    stop=True,
                )
                nc.scalar.activation(
                    out=h1_sb[:, m, :],
                    in_=pm[:, :],
                    func=mybir.ActivationFunctionType.Relu,
                    bias=b1_sb[:, m : m + 1],
                    scale=1.0,
                )

            # h2T[m] = relu(sum_k W2[k, :, m]^T @ h1T[k] + b2[m])
            h2_sb = consts.tile([P, K2, HS], f32, tag=f"h2{h}")
            for m in range(K2):
                pm = pmm.tile([P, HS], f32, tag="pm")
                for k in range(K1):
                    nc.tensor.matmul(
                        pm[:, :],
                        w2_sb[:, k, m * P : (m + 1) * P],
                        h1_sb[:, k, :],
                        start=(k == 0),
                        stop=(k == K1 - 1),
                    )
                nc.scalar.activation(
                    out=h2_sb[:, m, :],
                    in_=pm[:, :],
                    func=mybir.ActivationFunctionType.Relu,
                    bias=b2_sb[:, m : m + 1],
                    scale=1.0,
                )

            # out[:, n] = sum_k h2T[k]^T @ W3[k, :, n] + b3[:, n]
            # streamed per 512-column chunk through a small rotating tile
            for n in range(NT):
                po = pout.tile([HS, 512], f32, tag="po")
                for k in range(K2):
                    nc.tensor.matmul(
                        po[:, :],
                        h2_sb[:, k, :],
                        w3_sb[:, k, n * 512 : (n + 1) * 512],
                        start=(k == 0),
                        stop=(k == K2 - 1),
                    )
                ob = outp.tile([HS, 512], f32, tag="ob")
                nc.vector.tensor_add(
                    ob[:, :],
                    po[:, :],
                    b3_sb[h][:, n * 512 : (n + 1) * 512],
                )
                # SWDGE store: DMASW lanes unused by the feature stream.
                nc.gpsimd.dma_start(
                    out=out[h * HS : (h + 1) * HS, n * 512 : (n + 1) * 512],
                    in_=ob[:, :],
                )

        for s in range(SPC):
            reds = []
            for j in range(J):
                ft = fpool.tile([P, F], f32, tag="ft")
                nc.scalar.dma_start(out=ft, in_=fview[s, j])
                rj = redp.tile([P, RB * C], f32, tag="rj")
                chunk_tree(nc.vector, ft, None, rj)
                reds.append(rj)
                if j == 0:
                    # ACT observers: advance Act's DVE and Pool clocks past
                    # the tree reads of chunks 0-1 (one per engine), covering
                    # the slot releases the NEXT segment's reuse-DMAs
                    # (issued from Act) depend on -- they then wait only on
                    # their own DMA lane, and the pipeline never drains at
                    # segment boundaries.
                    nc.scalar.copy(
                        out=segobs[0:1, s : s + 1], in_=rj[0:1, 0:1]
                    )
            stride = 1
            while stride < J:
                for a in range(0, J, 2 * stride):
                    nc.vector.tensor_max(
                        reds[a][:, :], reds[a][:, :], reds[a + stride][:, :]
                    )
                stride *= 2
            if s == 0:
                # ACT observers for the bias lanes, emitted after segment
                # 0's feature DMAs so they never delay stream start; they
                # only need to precede the decoder relus.
                nc.scalar.copy(out=obs[0:1, 0:1], in_=b1_sb[0:1, 0:1])
                nc.scalar.copy(out=obs[0:1, 1:2], in_=b2_sb[0:1, 0:1])
                nc.scalar.copy(out=obs[0:1, 2:3], in_=b3_sb[0][0:1, 0:1])
            rs = reds[0]
            n = RB
            while n > 1:
                half = n // 2
                nc.vector.tensor_max(
                    rs[:, 0 : half * C],
                    rs[:, 0 : half * C],
                    rs[:, half * C : n * C],
                )
                n = half
            pt = ptr.tile([C, P], f32, tag="pt")
            nc.tensor.transpose(
                out=pt[:, :], in_=rs[:, 0:C], identity=ident[:, :]
            )
            nc.vector.reduce_max(out=gT[:, s : s + 1], in_=pt[:, :], axis=AX)
            if s == SPC // 2 - 1:
                decode_half(0)

        decode_half(1)
    nc.compile()
    _build_cache[cap] = nc
    return nc


def kernel(**inputs):
    global LAST_RESULTS
    features = np.ascontiguousarray(np.asarray(inputs["features"], dtype=np.float32))
    batch_ids = np.asarray(inputs["batch_ids"])
    W1 = np.ascontiguousarray(np.asarray(inputs["W1"], dtype=np.float32))
    b1 = np.asarray(inputs["b1"], dtype=np.float32)
    W2 = np.ascontiguousarray(np.asarray(inputs["W2"], dtype=np.float32))
    b2 = np.asarray(inputs["b2"], dtype=np.float32)
    W3 = np.ascontiguousarray(np.asarray(inputs["W3"], dtype=np.float32))
    b3 = np.asarray(inputs["b3"], dtype=np.float32)

    bounds = np.searchsorted(batch_ids, np.arange(B + 1), side="left")
    seg_len = np.diff(bounds)
    maxlen = max(1, int(seg_len.max()))
    L = -(-maxlen // P)  # ceil
    L = -(-L // J) * J  # round up to multiple of J
    L = max(L, 64)  # keep LQ >= 16 so the tree structure holds
    cap = L * P

    packed = np.empty((B, cap, C), np.float32)
    for b in range(B):
        lo, hi = int(bounds[b]), int(bounds[b + 1])
        n = hi - lo
        packed[b, :n] = features[lo:hi]
        packed[b, n:] = NEG

    b1t = np.ascontiguousarray(b1.reshape(K1, P).T)
    b2t = np.ascontiguousarray(b2.reshape(K2, P).T)
    b3r = np.ascontiguousarray(np.broadcast_to(b3, (SPC, OUT_D)))

    nc = _build(cap)

    in_maps = []
    for d in range(NCORES):
        in_maps.append(
            {
                "feats": packed[d * SPC : (d + 1) * SPC].reshape(SPC * cap, C),
                "w1": W1,
                "b1t": b1t,
                "w2": W2,
                "b2t": b2t,
                "w3": W3,
                "b3r": b3r,
            }
        )

    _ensure_axon_hooks()
    from concourse.bass_utils import run_bass_kernel_spmd

    core_ids = list(range(NCORES))
    try:
        res = run_bass_kernel_spmd(nc, in_maps, core_ids=core_ids)
    except Exception:
        if os.environ.get("BASS_TRACE") and not os.environ.get("BASS_NEVER_TRACE"):
            # trace post-processing can fail in restricted containers;
            # retry without tracing so the numeric result still lands.
            os.environ["BASS_NEVER_TRACE"] = "1"
            try:
                res = run_bass_kernel_spmd(nc, in_maps, core_ids=core_ids)
            finally:
                os.environ.pop("BASS_NEVER_TRACE", None)
        else:
            raise
    LAST_RESULTS = res

    full = np.concatenate([r["out"] for r in res.results], axis=0)
    return full.reshape(B, 3, NUM_POINTS)



# revision 4
# speedup vs baseline: 1.6209x; 1.6209x over previous
"""Trainium2 Bass kernel for nn_FCGFAutoencoder (segment_max -> 3-layer MLP decoder).

Strategy (data-parallel over segments, per sharding hint):
  - batch_ids are sorted, so the host finds the 65 segment boundaries with
    searchsorted and repacks features into a [B, cap, C] array, cast to
    fp16 (rel err ~3.6e-4 through the decoder, far under the 2e-2 gate),
    padded with -65504 (fp16 max-identity).  Each core gets 8 segments.
  - fp16 halves HBM traffic (32MB/core) AND doubles DVE tensor_tensor
    throughput (2x_1P packed mode), so the max-tree (~84us) hides under
    the DMA stream (~89us at the 360 GB/s per-core DMA-engine roofline).
  - The J=2 chunks of each segment stream on BOTH HWDGE queues (SP and
    Act) so queue-side gaps between consecutive DMAs on one ring are
    covered by the other ring (the baseline's single ring left the 16
    DMA engines idle ~29% of the time).
  - Weights/biases (cast to fp16 on host where they feed matmuls) load
    via the SWDGE ring (gpsimd), keeping both HWDGE rings free for the
    feature stream from t=0.
  - Per chunk: tensor_max tree [P, LQ*C] -> [P, RB*C]; combine chunks;
    final tree -> [P, C] fp16; cast to f32, PE-transpose, reduce -> gT.
  - Decoder (fp16 weights, f32 PSUM/biases) runs in two halves: half 0
    at the stream midpoint (hidden), half 1 as the only tail.
"""

import os
import sys
import types

sys.path.insert(0, "/opt/trn_rl_repo")

import numpy as np


def _ensure_axon_hooks():
    """Some images lack antenv.axon_hooks; bass_utils imports it when
    trace=True under axon. Install a shim that lazily wires the real
    ctypes-based NTFF hook from trn_agent_boot if present, else degrades
    to no-trace instead of crashing."""
    try:
        import antenv.axon_hooks  # noqa: F401

        return
    except ImportError:
        pass
    try:
        import antenv
    except ImportError:
        return
    mod = types.ModuleType("antenv.axon_hooks")
    _hook = [None]

    def set_axon_ntff_profile_hook(h):
        _hook[0] = h

    def get_axon_ntff_profile_hook():
        if _hook[0] is None:
            try:
                from trn_agent_boot.trn_boot import _ntff_profile_via_ctypes

                _hook[0] = _ntff_profile_via_ctypes("/opt/axon/libaxon_pjrt.so")
            except Exception:
                return None
        return _hook[0]

    mod.set_axon_ntff_profile_hook = set_axon_ntff_profile_hook
    mod.get_axon_ntff_profile_hook = get_axon_ntff_profile_hook
    sys.modules["antenv.axon_hooks"] = mod
    antenv.axon_hooks = mod

N = 4_194_304
C = 32
B = 64
NUM_POINTS = 1024
NCORES = 8
SPC = B // NCORES  # segments per core
P = 128
J = 2  # DMA chunks per segment (one per HWDGE queue)
NEG = -65504.0  # fp16 lowest: max-identity padding
H1, H2, OUT_D = 256, 512, 3 * NUM_POINTS
K1, K2, NT = H1 // P, H2 // P, OUT_D // 512

LAST_RESULTS = None

_build_cache = {}


def _build(cap):
    if cap in _build_cache:
        return _build_cache[cap]

    import concourse.bacc as bacc
    import concourse.tile as tile
    from concourse import mybir
    from concourse.masks import make_identity
    from contextlib import ExitStack

    L = cap // P  # rows per partition per segment
    LQ = L // J  # rows per partition per DMA chunk
    F = LQ * C  # free elems per chunk tile

    f32 = mybir.dt.float32
    f16 = mybir.dt.float16
    AX = mybir.AxisListType.X
    nc = bacc.Bacc("TRN2", target_bir_lowering=False)

    feats = nc.dram_tensor("feats", [SPC * cap, C], f16, kind="ExternalInput")
    w1 = nc.dram_tensor("w1", [C, H1], f16, kind="ExternalInput")
    b1t = nc.dram_tensor("b1t", [P, K1], f32, kind="ExternalInput")
    w2 = nc.dram_tensor("w2", [H1, H2], f16, kind="ExternalInput")
    b2t = nc.dram_tensor("b2t", [P, K2], f32, kind="ExternalInput")
    w3 = nc.dram_tensor("w3", [H2, OUT_D], f16, kind="ExternalInput")
    b3r = nc.dram_tensor("b3r", [SPC, OUT_D], f32, kind="ExternalInput")
    out = nc.dram_tensor("out", [SPC, OUT_D], f32, kind="ExternalOutput")

    # rows: s*cap + p*L + j*LQ + i  ->  [s, j, p, (i c)]
    fview = feats[:].rearrange("(s p j i) c -> s j p (i c)", s=SPC, p=P, j=J)

    with ExitStack() as ctx:
        tc = ctx.enter_context(tile.TileContext(nc))
        consts = ctx.enter_context(tc.tile_pool(name="consts", bufs=1))
        fpool = ctx.enter_context(tc.tile_pool(name="feat", bufs=3))  # x J tags
        outp = ctx.enter_context(tc.tile_pool(name="outp", bufs=2))
        redp = ctx.enter_context(tc.tile_pool(name="red", bufs=2 * J))
        ptr = ctx.enter_context(tc.tile_pool(name="ptr", bufs=2, space="PSUM"))
        pmm = ctx.enter_context(tc.tile_pool(name="pmm", bufs=2, space="PSUM"))
        pout = ctx.enter_context(tc.tile_pool(name="pout", bufs=2, space="PSUM"))

        ident = consts.tile([P, P], f32)
        make_identity(nc, ident)

        # All weight/bias loads ride the SWDGE ring (gpsimd) so BOTH
        # HWDGE rings (SP + Act) are free for the feature stream from
        # t=0.  SWDGE is FIFO and otherwise idle until the output
        # stores at decode time.
        b1_sb = consts.tile([P, K1], f32)
        nc.gpsimd.dma_start(out=b1_sb, in_=b1t[:])
        b2_sb = consts.tile([P, K2], f32)
        nc.gpsimd.dma_start(out=b2_sb, in_=b2t[:])
        HS = SPC // 2  # segments per decoder half
        b3_sb = []
        for h in range(2):
            bh = consts.tile([HS, OUT_D], f32, tag=f"b3h{h}")
            nc.gpsimd.dma_start(out=bh, in_=b3r[h * HS : (h + 1) * HS])
            b3_sb.append(bh)
        w1_sb = consts.tile([C, H1], f16)
        nc.gpsimd.dma_start(out=w1_sb, in_=w1[:])
        w2_sb = consts.tile([P, K1, H2], f16)
        nc.gpsimd.dma_start(out=w2_sb, in_=w2[:].rearrange("(k p) n -> p k n", p=P))
        w3_sb = consts.tile([P, K2, OUT_D], f16)
        nc.gpsimd.dma_start(out=w3_sb, in_=w3[:].rearrange("(k p) n -> p k n", p=P))

        # Observer copies: each engine may carry only ONE semaphore wait per
        # instruction, so advance ACT's and DVE's vector clocks over the
        # bias/weight SWDGE lane early; the decoder relu/add ops then need
        # only their PE wait.  (All SWDGE loads share one FIFO lane, so one
        # observer per engine past the LAST load covers them all.)
        obs = consts.tile([1, 16], f32)
        nc.vector.tensor_copy(out=obs[0:1, 3:4], in_=w3_sb[0:1, 0, 0:1])

        # PE (Matmult/LDW) supports only ONE sync wait per instruction, so a
        # matmul whose inputs come from two unobserved semaphores fails to
        # compile. Prime PE with throwaway single-wait transposes so it has
        # observed the identity (Pool) and the SWDGE weight lane before the
        # real matmuls. Each gets its own PSUM slot (slot reuse would add a
        # second, PE-release wait); the pool closes before the others open.
        with tc.tile_pool(name="prime", bufs=1, space="PSUM") as primep:
            pp = primep.tile([C, P], f32, tag="prime")
            nc.tensor.transpose(
                out=pp[0:C, 0:P], in_=ident[:, 0:C], identity=ident[:, :]
            )
            # fp16 matmul with both operands from the SWDGE lane: one wait.
            pp2 = primep.tile([1, P], f32, tag="prime16")
            nc.tensor.matmul(
                pp2[0:1, 0:C],
                w3_sb[:, 0, 0:1],
                w3_sb[:, 0, 0:C],
                start=True,
                stop=True,
            )

        gT = consts.tile([C, SPC], f32)
        segobs = consts.tile([1, SPC], f32)

        RB = 8  # row-blocks kept per chunk; small levels are overhead-bound

        def chunk_tree(eng, ft, rj):
            # contiguous tree max over the row axis: pairs (i, c) with
            # (i + n/2, c); in-place halving within ft. Stops at RB
            # blocks (tail levels are fixed-overhead-dominated); rj is
            # [P, RB*C] and the cross-chunk combine finishes the job.
            cur = ft
            n = LQ
            while n > 2 * RB:
                if n % 2 == 1:
                    eng.tensor_max(
                        cur[:, 0:C], cur[:, 0:C], cur[:, (n - 1) * C : n * C]
                    )
                    n -= 1
                half = n // 2
                eng.tensor_max(
                    cur[:, 0 : half * C],
                    cur[:, 0 : half * C],
                    cur[:, half * C : n * C],
                )
                n = half
            while n % RB:
                eng.tensor_max(cur[:, 0:C], cur[:, 0:C], cur[:, (n - 1) * C : n * C])
                n -= 1
            eng.tensor_max(
                rj[:, :], cur[:, 0 : (n // 2) * C], cur[:, (n // 2) * C : n * C]
            )

        def decode_half(h):
            # decoder for segments [h*HS, (h+1)*HS): runs while the other
            # half is still streaming, so only the last half is tail time.
            cols = slice(h * HS, (h + 1) * HS)
            # empty segments: reference maps -inf -> 0; padding is -65504,
            # so mask = (g > -60000) in {0,1}; g * mask zeroes empties.
            mask = consts.tile([C, HS], f32, tag=f"mask{h}")
            gfix = consts.tile([C, HS], f32, tag=f"gfix{h}")
            nc.vector.tensor_scalar(
                out=mask[:, :],
                in0=gT[:, cols],
                scalar1=-60000.0,
                scalar2=None,
                op0=mybir.AluOpType.is_gt,
            )
            nc.vector.tensor_mul(gfix[:, :], gT[:, cols], mask[:, :])
            g16 = consts.tile([C, HS], f16, tag=f"g16{h}")
            nc.vector.tensor_copy(out=g16[:, :], in_=gfix[:, :])

            # h1T[m] = relu(W1[:, m]^T @ g + b1[m])   [128, HS] per chunk m
            h1_sb = consts.tile([P, K1, HS], f16, tag=f"h1{h}")
            for m in range(K1):
                pm = pmm.tile([P, HS], f32, tag="pm")
                nc.tensor.matmul(
                    pm[:, :],
                    w1_sb[:, m * P : (m + 1) * P],
                    g16[:, :],
                    start=True,
                    stop=True,
                )
                nc.scalar.activation(
                    out=h1_sb[:, m, :],
                    in_=pm[:, :],
                    func=mybir.ActivationFunctionType.Relu,
                    bias=b1_sb[:, m : m + 1],
                    scale=1.0,
                )

            # h2T[m] = relu(sum_k W2[k, :, m]^T @ h1T[k] + b2[m])
            h2_sb = consts.tile([P, K2, HS], f16, tag=f"h2{h}")
            for m in range(K2):
                pm = pmm.tile([P, HS], f32, tag="pm")
                for k in range(K1):
                    nc.tensor.matmul(
                        pm[:, :],
                        w2_sb[:, k, m * P : (m + 1) * P],
                        h1_sb[:, k, :],
                        start=(k == 0),
                        stop=(k == K1 - 1),
                    )
                nc.scalar.activation(
                    out=h2_sb[:, m, :],
                    in_=pm[:, :],
                    func=mybir.ActivationFunctionType.Relu,
                    bias=b2_sb[:, m : m + 1],
                    scale=1.0,
                )

            # out[:, n] = sum_k h2T[k]^T @ W3[k, :, n] + b3[:, n]
            # streamed per 512-column chunk through a small rotating tile
            for n in range(NT):
                po = pout.tile([HS, 512], f32, tag="po")
                for k in range(K2):
                    nc.tensor.matmul(
                        po[:, :],
                        h2_sb[:, k, :],
                        w3_sb[:, k, n * 512 : (n + 1) * 512],
                        start=(k == 0),
                        stop=(k == K2 - 1),
                    )
                ob = outp.tile([HS, 512], f32, tag="ob")
                nc.vector.tensor_add(
                    ob[:, :],
                    po[:, :],
                    b3_sb[h][:, n * 512 : (n + 1) * 512],
                )
                # SWDGE store: DMASW lanes unused by the feature stream.
                nc.gpsimd.dma_start(
                    out=out[h * HS : (h + 1) * HS, n * 512 : (n + 1) * 512],
                    in_=ob[:, :],
                )

        qeng = [nc.sync, nc.scalar]  # one HWDGE ring per chunk
        for s in range(SPC):
            reds = []
            for j in range(J):
                ft = fpool.tile([P, F], f16, tag=f"ft{j}")
                qeng[j].dma_start(out=ft, in_=fview[s, j])
                rj = redp.tile([P, RB * C], f16, tag=f"rj{j}")
                chunk_tree(nc.vector, ft, rj)
                reds.append(rj)
                if j == 0:
                    # ACT observer: advance Act's DVE clock past the tree
                    # reads of this segment's chunks, covering the slot
                    # releases the NEXT segments' Act-ring reuse-DMAs
                    # depend on -- they then wait only on their own DMA
                    # lane, and the pipeline never drains at segment
                    # boundaries.
                    nc.scalar.copy(
                        out=segobs[0:1, s : s + 1], in_=rj[0:1, 0:1]
                    )
            # cross-chunk combine (J=2)
            nc.vector.tensor_max(reds[0][:, :], reds[0][:, :], reds[1][:, :])
            if s == 0:
                # ACT observer for the SWDGE weight/bias lane, emitted
                # after segment 0's feature DMAs so it never delays
                # stream start; it only needs to precede the decoder.
                nc.scalar.copy(out=obs[0:1, 0:1], in_=w3_sb[0:1, 0, 0:1])
            rs = reds[0]
            n = RB
            while n > 1:
                half = n // 2
                nc.vector.tensor_max(
                    rs[:, 0 : half * C],
                    rs[:, 0 : half * C],
                    rs[:, half * C : n * C],
                )
                n = half
            rs32 = redp.tile([P, C], f32, tag="rs32")
            nc.vector.tensor_copy(out=rs32[:, :], in_=rs[:, 0:C])
            pt = ptr.tile([C, P], f32, tag="pt")
            nc.tensor.transpose(
                out=pt[:, :], in_=rs32[:, :], identity=ident[:, :]
            )
            nc.vector.reduce_max(out=gT[:, s : s + 1], in_=pt[:, :], axis=AX)
            if s == SPC // 2 - 1:
                decode_half(0)

        decode_half(1)
    nc.compile()
    _build_cache[cap] = nc
    return nc


def kernel(**inputs):
    global LAST_RESULTS
    features = np.asarray(inputs["features"], dtype=np.float32)
    batch_ids = np.asarray(inputs["batch_ids"])
    W1 = np.asarray(inputs["W1"], dtype=np.float32)
    b1 = np.asarray(inputs["b1"], dtype=np.float32)
    W2 = np.asarray(inputs["W2"], dtype=np.float32)
    b2 = np.asarray(inputs["b2"], dtype=np.float32)
    W3 = np.asarray(inputs["W3"], dtype=np.float32)
    b3 = np.asarray(inputs["b3"], dtype=np.float32)

    bounds = np.searchsorted(batch_ids, np.arange(B + 1), side="left")
    seg_len = np.diff(bounds)
    maxlen = max(1, int(seg_len.max()))
    L = -(-maxlen // P)  # ceil
    L = -(-L // J) * J  # round up to multiple of J
    L = max(L, 32)  # keep LQ >= 16 so the tree structure holds
    cap = L * P

    packed = np.empty((B, cap, C), np.float16)
    for b in range(B):
        lo, hi = int(bounds[b]), int(bounds[b + 1])
        n = hi - lo
        packed[b, :n] = features[lo:hi]
        packed[b, n:] = NEG

    w1h = np.ascontiguousarray(W1.astype(np.float16))
    w2h = np.ascontiguousarray(W2.astype(np.float16))
    w3h = np.ascontiguousarray(W3.astype(np.float16))
    b1t = np.ascontiguousarray(b1.reshape(K1, P).T)
    b2t = np.ascontiguousarray(b2.reshape(K2, P).T)
    b3r = np.ascontiguousarray(np.broadcast_to(b3, (SPC, OUT_D)))

    nc = _build(cap)

    in_maps = []
    for d in range(NCORES):
        in_maps.append(
            {
                "feats": packed[d * SPC : (d + 1) * SPC].reshape(SPC * cap, C),
                "w1": w1h,
                "b1t": b1t,
                "w2": w2h,
                "b2t": b2t,
                "w3": w3h,
                "b3r": b3r,
            }
        )

    _ensure_axon_hooks()
    from concourse.bass_utils import run_bass_kernel_spmd

    core_ids = list(range(NCORES))
    try:
        res = run_bass_kernel_spmd(nc, in_maps, core_ids=core_ids)
    except Exception:
        if os.environ.get("BASS_TRACE") and not os.environ.get("BASS_NEVER_TRACE"):
            # trace post-processing can fail in restricted containers;
            # retry without tracing so the numeric result still lands.
            os.environ["BASS_NEVER_TRACE"] = "1"
            try:
                res = run_bass_kernel_spmd(nc, in_maps, core_ids=core_ids)
            finally:
                os.environ.pop("BASS_NEVER_TRACE", None)
        else:
            raise
    LAST_RESULTS = res

    full = np.concatenate([r["out"] for r in res.results], axis=0)
    return full.reshape(B, 3, NUM_POINTS)


# revision 7
# speedup vs baseline: 1.6382x; 1.0106x over previous
"""Trainium2 Bass kernel for nn_FCGFAutoencoder (segment_max -> 3-layer MLP decoder).

Strategy (data-parallel over segments, per sharding hint):
  - batch_ids are sorted, so the host finds the 65 segment boundaries with
    searchsorted and repacks features into a [B, cap, C] array, cast to
    fp16 (rel err ~3.6e-4 through the decoder, far under the 2e-2 gate),
    padded with -65504 (fp16 max-identity).  Each core gets 8 segments.
  - fp16 halves HBM traffic (32MB/core) AND doubles DVE tensor_tensor
    throughput (2x_1P packed mode), so the max-tree (~84us) hides under
    the DMA stream (~89us at the 360 GB/s per-core DMA-engine roofline).
  - The J=2 chunks of each segment stream on BOTH HWDGE queues (SP and
    Act) so queue-side gaps between consecutive DMAs on one ring are
    covered by the other ring (the baseline's single ring left the 16
    DMA engines idle ~29% of the time).
  - Weights/biases (cast to fp16 on host where they feed matmuls) load
    via the SWDGE ring (gpsimd), keeping both HWDGE rings free for the
    feature stream from t=0.
  - Per chunk: tensor_max tree [P, LQ*C] -> [P, RB*C]; combine chunks;
    final tree -> [P, C] fp16; cast to f32, PE-transpose, reduce -> gT.
  - Decoder (fp16 weights, f32 PSUM/biases) runs in two halves: half 0
    at the stream midpoint (hidden), half 1 as the only tail.
"""

import os
import sys
import types

sys.path.insert(0, "/opt/trn_rl_repo")

import numpy as np


def _ensure_axon_hooks():
    """Some images lack antenv.axon_hooks; bass_utils imports it when
    trace=True under axon. Install a shim that lazily wires the real
    ctypes-based NTFF hook from trn_agent_boot if present, else degrades
    to no-trace instead of crashing."""
    try:
        import antenv.axon_hooks  # noqa: F401

        return
    except ImportError:
        pass
    try:
        import antenv
    except ImportError:
        return
    mod = types.ModuleType("antenv.axon_hooks")
    _hook = [None]

    def set_axon_ntff_profile_hook(h):
        _hook[0] = h

    def get_axon_ntff_profile_hook():
        if _hook[0] is None:
            try:
                from trn_agent_boot.trn_boot import _ntff_profile_via_ctypes

                _hook[0] = _ntff_profile_via_ctypes("/opt/axon/libaxon_pjrt.so")
            except Exception:
                return None
        return _hook[0]

    mod.set_axon_ntff_profile_hook = set_axon_ntff_profile_hook
    mod.get_axon_ntff_profile_hook = get_axon_ntff_profile_hook
    sys.modules["antenv.axon_hooks"] = mod
    antenv.axon_hooks = mod

N = 4_194_304
C = 32
B = 64
NUM_POINTS = 1024
NCORES = 8
SPC = B // NCORES  # segments per core
P = 128
J = 2  # DMA chunks per segment (one per HWDGE queue)
NEG = -65504.0  # fp16 lowest: max-identity padding
H1, H2, OUT_D = 256, 512, 3 * NUM_POINTS
K1, K2, NT = H1 // P, H2 // P, OUT_D // 512

LAST_RESULTS = None

_build_cache = {}


def _build(cap):
    if cap in _build_cache:
        return _build_cache[cap]

    import concourse.bacc as bacc
    import concourse.tile as tile
    from concourse import mybir
    from concourse.masks import make_identity
    from contextlib import ExitStack

    L = cap // P  # rows per partition per segment
    LQ = L // J  # rows per partition per DMA chunk
    F = LQ * C  # free elems per chunk tile

    f32 = mybir.dt.float32
    f16 = mybir.dt.float16
    AX = mybir.AxisListType.X
    nc = bacc.Bacc("TRN2", target_bir_lowering=False)

    feats = nc.dram_tensor("feats", [SPC * cap, C], f16, kind="ExternalInput")
    w1 = nc.dram_tensor("w1", [C, H1], f16, kind="ExternalInput")
    b1t = nc.dram_tensor("b1t", [P, K1], f32, kind="ExternalInput")
    w2 = nc.dram_tensor("w2", [H1, H2], f16, kind="ExternalInput")
    b2t = nc.dram_tensor("b2t", [P, K2], f32, kind="ExternalInput")
    w3 = nc.dram_tensor("w3", [H2, OUT_D], f16, kind="ExternalInput")
    b3r = nc.dram_tensor("b3r", [SPC, OUT_D], f32, kind="ExternalInput")
    out = nc.dram_tensor("out", [SPC, OUT_D], f32, kind="ExternalOutput")

    # rows: s*cap + p*L + j*LQ + i  ->  [s, j, p, (i c)]
    fview = feats[:].rearrange("(s p j i) c -> s j p (i c)", s=SPC, p=P, j=J)

    with ExitStack() as ctx:
        tc = ctx.enter_context(tile.TileContext(nc))
        consts = ctx.enter_context(tc.tile_pool(name="consts", bufs=1))
        fpool = ctx.enter_context(tc.tile_pool(name="feat", bufs=3))  # x J tags
        outp = ctx.enter_context(tc.tile_pool(name="outp", bufs=2))
        redp = ctx.enter_context(tc.tile_pool(name="red", bufs=2 * J))
        ptr = ctx.enter_context(tc.tile_pool(name="ptr", bufs=2, space="PSUM"))
        pmm = ctx.enter_context(tc.tile_pool(name="pmm", bufs=2, space="PSUM"))
        pout = ctx.enter_context(tc.tile_pool(name="pout", bufs=2, space="PSUM"))

        ident = consts.tile([P, P], f32)
        make_identity(nc, ident)

        # Weight/bias tiles: DMAs are emitted AFTER segment 0's feature
        # DMAs (see the segment loop) so the 3.4MB of weights streams
        # BEHIND segment 0 through the shared DMA engines instead of
        # delaying the first tree by ~8us.  They ride the SP ring, whose
        # FIFO then naturally interleaves them between segment 0 and
        # segment 2 (features alternate SP/Act per segment).
        b1_sb = consts.tile([P, K1], f32)
        b2_sb = consts.tile([P, K2], f32)
        HS = SPC // 2  # segments per decoder half
        b3_sb = [
            consts.tile([HS, OUT_D], f32, tag=f"b3h{h}", name=f"b3h{h}")
            for h in range(2)
        ]
        w1_sb = consts.tile([C, H1], f16)
        w2_sb = consts.tile([P, K1, H2], f16)
        w3_sb = consts.tile([P, K2, OUT_D], f16)

        def load_weights():
            nc.sync.dma_start(out=b1_sb, in_=b1t[:])
            nc.sync.dma_start(out=b2_sb, in_=b2t[:])
            for h in range(2):
                nc.sync.dma_start(out=b3_sb[h], in_=b3r[h * HS : (h + 1) * HS])
            nc.sync.dma_start(out=w1_sb, in_=w1[:])
            nc.sync.dma_start(
                out=w2_sb, in_=w2[:].rearrange("(k p) n -> p k n", p=P)
            )
            nc.sync.dma_start(
                out=w3_sb, in_=w3[:].rearrange("(k p) n -> p k n", p=P)
            )

        obs = consts.tile([1, 16], f32)
        gT = consts.tile([C, SPC], f32)
        segobs = consts.tile([1, SPC], f32)

        RB = 8  # row-blocks kept per chunk; small levels are overhead-bound

        def chunk_tree(eng, ft, rj):
            # contiguous tree max over the row axis: pairs (i, c) with
            # (i + n/2, c); in-place halving within ft. Stops at RB
            # blocks (tail levels are fixed-overhead-dominated); rj is
            # [P, RB*C] and the cross-chunk combine finishes the job.
            cur = ft
            n = LQ
            while n > 2 * RB:
                if n % 2 == 1:
                    eng.tensor_max(
                        cur[:, 0:C], cur[:, 0:C], cur[:, (n - 1) * C : n * C]
                    )
                    n -= 1
                half = n // 2
                eng.tensor_max(
                    cur[:, 0 : half * C],
                    cur[:, 0 : half * C],
                    cur[:, half * C : n * C],
                )
                n = half
            while n % RB:
                eng.tensor_max(cur[:, 0:C], cur[:, 0:C], cur[:, (n - 1) * C : n * C])
                n -= 1
            eng.tensor_max(
                rj[:, :], cur[:, 0 : (n // 2) * C], cur[:, (n // 2) * C : n * C]
            )

        def decode_half(h):
            # decoder for segments [h*HS, (h+1)*HS): runs while the other
            # half is still streaming, so only the last half is tail time.
            cols = slice(h * HS, (h + 1) * HS)
            # empty segments: reference maps -inf -> 0; padding is -65504,
            # so mask = (g > -60000) in {0,1}; g * mask zeroes empties.
            mask = consts.tile([C, HS], f32, tag=f"mask{h}")
            gfix = consts.tile([C, HS], f32, tag=f"gfix{h}")
            nc.vector.tensor_scalar(
                out=mask[:, :],
                in0=gT[:, cols],
                scalar1=-60000.0,
                scalar2=None,
                op0=mybir.AluOpType.is_gt,
            )
            nc.vector.tensor_mul(gfix[:, :], gT[:, cols], mask[:, :])
            g16 = consts.tile([C, HS], f16, tag=f"g16{h}")
            nc.vector.tensor_copy(out=g16[:, :], in_=gfix[:, :])

            # h1T[m] = relu(W1[:, m]^T @ g + b1[m])   [128, HS] per chunk m
            h1_sb = consts.tile([P, K1, HS], f16, tag=f"h1{h}")
            for m in range(K1):
                pm = pmm.tile([P, HS], f32, tag="pm")
                nc.tensor.matmul(
                    pm[:, :],
                    w1_sb[:, m * P : (m + 1) * P],
                    g16[:, :],
                    start=True,
                    stop=True,
                )
                nc.scalar.activation(
                    out=h1_sb[:, m, :],
                    in_=pm[:, :],
                    func=mybir.ActivationFunctionType.Relu,
                    bias=b1_sb[:, m : m + 1],
                    scale=1.0,
                )

            # h2T[m] = relu(sum_k W2[k, :, m]^T @ h1T[k] + b2[m])
            h2_sb = consts.tile([P, K2, HS], f16, tag=f"h2{h}")
            for m in range(K2):
                pm = pmm.tile([P, HS], f32, tag="pm")
                for k in range(K1):
                    nc.tensor.matmul(
                        pm[:, :],
                        w2_sb[:, k, m * P : (m + 1) * P],
                        h1_sb[:, k, :],
                        start=(k == 0),
                        stop=(k == K1 - 1),
                    )
                nc.scalar.activation(
                    out=h2_sb[:, m, :],
                    in_=pm[:, :],
                    func=mybir.ActivationFunctionType.Relu,
                    bias=b2_sb[:, m : m + 1],
                    scale=1.0,
                )

            # out[:, n] = sum_k h2T[k]^T @ W3[k, :, n] + b3[:, n]
            # streamed per 512-column chunk through a small rotating tile
            for n in range(NT):
                po = pout.tile([HS, 512], f32, tag="po")
                for k in range(K2):
                    nc.tensor.matmul(
                        po[:, :],
                        h2_sb[:, k, :],
                        w3_sb[:, k, n * 512 : (n + 1) * 512],
                        start=(k == 0),
                        stop=(k == K2 - 1),
                    )
                ob = outp.tile([HS, 512], f32, tag="ob")
                nc.vector.tensor_add(
                    ob[:, :],
                    po[:, :],
                    b3_sb[h][:, n * 512 : (n + 1) * 512],
                )
                # SWDGE store: DMASW lanes unused by the feature stream.
                nc.gpsimd.dma_start(
                    out=out[h * HS : (h + 1) * HS, n * 512 : (n + 1) * 512],
                    in_=ob[:, :],
                )

        # Both chunks of segment s ride ONE HWDGE ring, alternating
        # rings per segment: each ring then has a whole 2-segment period
        # (~24us) to retrigger its next DMA, so trigger/semaphore
        # latency never leaves the shared DMA engines idle.
        qeng = [nc.sync, nc.scalar]
        for s in range(SPC):
            q = qeng[s % 2]
            reds = []
            for j in range(J):
                ft = fpool.tile([P, F], f16, tag=f"ft{j}")
                q.dma_start(out=ft, in_=fview[s, j])
                rj = redp.tile([P, RB * C], f16, tag=f"rj{j}")
                chunk_tree(nc.vector, ft, rj)
                reds.append(rj)
                if j == 0:
                    # ACT observer: advance Act's DVE clock past the tree
                    # reads of this segment's chunks, covering the slot
                    # releases the NEXT segments' Act-ring reuse-DMAs
                    # depend on -- they then wait only on their own DMA
                    # lane, and the pipeline never drains at segment
                    # boundaries.
                    nc.scalar.copy(
                        out=segobs[0:1, s : s + 1], in_=rj[0:1, 0:1]
                    )
            if s == 0:
                # Weights enter the SP FIFO here -- after segment 0's
                # chunks, before segment 2's -- so they stream behind
                # the first segment instead of ahead of it.
                load_weights()
                # PE (Matmult/LDW) supports only ONE sync wait per
                # instruction, so a matmul whose inputs come from two
                # unobserved semaphores fails to compile. Prime PE with
                # throwaway single-wait ops so it has observed the
                # identity (Pool lane) and the SP weight lane before the
                # real matmuls. Each gets its own PSUM slot (slot reuse
                # would add a second, PE-release wait); the pool closes
                # before first use of the others.
                with tc.tile_pool(name="prime", bufs=1, space="PSUM") as primep:
                    pp = primep.tile([C, P], f32, tag="prime")
                    nc.tensor.transpose(
                        out=pp[0:C, 0:P], in_=ident[:, 0:C], identity=ident[:, :]
                    )
                    # fp16 matmul, both operands from the SP weight lane.
                    pp2 = primep.tile([1, P], f32, tag="prime16")
                    nc.tensor.matmul(
                        pp2[0:1, 0:C],
                        w3_sb[:, 0, 0:1],
                        w3_sb[:, 0, 0:C],
                        start=True,
                        stop=True,
                    )
                # Observers: advance ACT's and DVE's clocks over the SP
                # weight lane (w3 is the LAST weight DMA in the FIFO, so
                # one observer per engine covers all weight/bias loads);
                # decoder relu/add ops then need only their PE wait.
                nc.scalar.copy(out=obs[0:1, 0:1], in_=w3_sb[0:1, 0, 0:1])
                nc.vector.tensor_copy(out=obs[0:1, 3:4], in_=w3_sb[0:1, 0, 0:1])
            # cross-chunk combine (J=2)
            nc.vector.tensor_max(reds[0][:, :], reds[0][:, :], reds[1][:, :])
            rs = reds[0]
            n = RB
            while n > 1:
                half = n // 2
                nc.vector.tensor_max(
                    rs[:, 0 : half * C],
                    rs[:, 0 : half * C],
                    rs[:, half * C : n * C],
                )
                n = half
            rs32 = redp.tile([P, C], f32, tag="rs32")
            nc.vector.tensor_copy(out=rs32[:, :], in_=rs[:, 0:C])
            pt = ptr.tile([C, P], f32, tag="pt")
            nc.tensor.transpose(
                out=pt[:, :], in_=rs32[:, :], identity=ident[:, :]
            )
            nc.vector.reduce_max(out=gT[:, s : s + 1], in_=pt[:, :], axis=AX)
            if s == SPC // 2 - 1:
                decode_half(0)

        decode_half(1)
    nc.compile()
    _build_cache[cap] = nc
    return nc


def kernel(**inputs):
    global LAST_RESULTS
    features = np.asarray(inputs["features"], dtype=np.float32)
    batch_ids = np.asarray(inputs["batch_ids"])
    W1 = np.asarray(inputs["W1"], dtype=np.float32)
    b1 = np.asarray(inputs["b1"], dtype=np.float32)
    W2 = np.asarray(inputs["W2"], dtype=np.float32)
    b2 = np.asarray(inputs["b2"], dtype=np.float32)
    W3 = np.asarray(inputs["W3"], dtype=np.float32)
    b3 = np.asarray(inputs["b3"], dtype=np.float32)

    bounds = np.searchsorted(batch_ids, np.arange(B + 1), side="left")
    seg_len = np.diff(bounds)
    maxlen = max(1, int(seg_len.max()))
    L = -(-maxlen // P)  # ceil
    L = -(-L // J) * J  # round up to multiple of J
    L = max(L, 32)  # keep LQ >= 16 so the tree structure holds
    cap = L * P

    packed = np.empty((B, cap, C), np.float16)
    for b in range(B):
        lo, hi = int(bounds[b]), int(bounds[b + 1])
        n = hi - lo
        packed[b, :n] = features[lo:hi]
        packed[b, n:] = NEG

    w1h = np.ascontiguousarray(W1.astype(np.float16))
    w2h = np.ascontiguousarray(W2.astype(np.float16))
    w3h = np.ascontiguousarray(W3.astype(np.float16))
    b1t = np.ascontiguousarray(b1.reshape(K1, P).T)
    b2t = np.ascontiguousarray(b2.reshape(K2, P).T)
    b3r = np.ascontiguousarray(np.broadcast_to(b3, (SPC, OUT_D)))

    nc = _build(cap)

    in_maps = []
    for d in range(NCORES):
        in_maps.append(
            {
                "feats": packed[d * SPC : (d + 1) * SPC].reshape(SPC * cap, C),
                "w1": w1h,
                "b1t": b1t,
                "w2": w2h,
                "b2t": b2t,
                "w3": w3h,
                "b3r": b3r,
            }
        )

    _ensure_axon_hooks()
    from concourse.bass_utils import run_bass_kernel_spmd

    core_ids = list(range(NCORES))
    try:
        res = run_bass_kernel_spmd(nc, in_maps, core_ids=core_ids)
    except Exception:
        if os.environ.get("BASS_TRACE") and not os.environ.get("BASS_NEVER_TRACE"):
            # trace post-processing can fail in restricted containers;
            # retry without tracing so the numeric result still lands.
            os.environ["BASS_NEVER_TRACE"] = "1"
            try:
                res = run_bass_kernel_spmd(nc, in_maps, core_ids=core_ids)
            finally:
                os.environ.pop("BASS_NEVER_TRACE", None)
        else:
            raise
    LAST_RESULTS = res

    full = np.concatenate([r["out"] for r in res.results], axis=0)
    return full.reshape(B, 3, NUM_POINTS)
